# revision 16
# baseline (speedup 1.0000x reference)
"""Self-contained Trainium2 Bass kernel for single-head full-dim attention.

Reference computation (fp32 jax):
    q  = x @ Wq                      # [B, Nq, D]
    kv = y @ Wkv                     # [B, Nkv, 2D] -> k, v
    attn = softmax(q * D^-0.5 @ k^T) # [B, Nq, Nkv]
    out  = attn @ v                  # [B, Nq, D]
with B=4, Nq=Nkv=2048, D=1024.

Distribution: data parallel over 8 NeuronCores, shard = (batch b,
kv-half s).  Each core handles all 2048 queries of its batch against
its 1024 keys, producing the UNNORMALIZED output block
out'_s = exp(S_s) @ v_s and the partial softmax denominator
Z_s = sum_k exp(S_s).  The host combines the two halves:
out = (out'_0 + out'_1) / (Z_0 + Z_1).  No collectives.

Key algebraic trick: the Q and K projections are folded into a single
host-precomputed matrix M = (Wq * D^-0.5) @ Wk^T, so
    scores = (x @ M) @ y^T.
This removes the K projection entirely and de-duplicates the Q
projection across the core pair "for free": per-core flops hit the
ideal total/8 (15.0 GF vs 17.2 GF for the q/k-projection layout).

Layouts: everything on-chip is computed transposed ([feature, token])
so the TensorEngine contracts along partitions without any on-chip
transposes; the unnormalized output is produced transposed
([d_out, nq]) which lets the stationary operand be a v-tile and the
moving operand stream full 512-wide nq chunks.  All matmul operands
bf16 (fp32 PSUM accumulation); the out' result is stored bf16 (the
2e-2 rel tolerance dwarfs bf16 rounding).  exp runs without
max-subtraction (scores ~ N(0,1) by construction) on the scalar
engine; Z is a ones-vector matmul emitted after P7 so its serial
chain hides behind the output-DMA drain.
"""

import numpy as np
import ml_dtypes

import concourse.bass as bass
import concourse.mybir as mybir
import concourse.tile as tile
from concourse.bass import ds
from concourse.bass_utils import run_bass_kernel_spmd

DIM = 1024
B = 4
NQ = 2048
NKV = 2048
N_CORES = 8
NKV_SHARD = 1024  # keys per core

BF16 = mybir.dt.bfloat16
F32 = mybir.dt.float32
NP_BF16 = ml_dtypes.bfloat16


def _split_sync_waits(nc, max_waits: int = 1):
    """walrus in this toolchain rejects instructions carrying more than one
    sem wait ("Too many sync wait commands").  Hoist extra waits onto
    preceding same-engine NOPs: the engine dispatches in order, so waiting
    just before the instruction is semantically identical (at worst it
    delays issue slightly)."""
    import bass_rust as _bass_rust

    for f in nc.m.functions:
        for bb in f.blocks:
            insts = list(bb.instructions)
            out = []
            changed = False
            for inst in insts:
                si = getattr(inst, "sync_info", None)
                waits = list(si.on_wait) if si is not None and si.on_wait else []
                if len(waits) > max_waits:
                    changed = True
                    extra, keep = waits[:-max_waits], waits[-max_waits:]
                    for k in range(0, len(extra), max_waits):
                        nop = mybir.InstNoOp(
                            name=f"{inst.name}_sw{k}", engine=inst.engine,
                            ins=[], outs=[],
                        )
                        nop.sync_info = _bass_rust.SyncInfo(
                            on_wait=extra[k : k + max_waits], on_update=[]
                        )
                        out.append(nop)
                    si.on_wait = keep
                    inst.sync_info = si
                out.append(inst)
            if changed:
                bb.instructions = out


def build_attention_nc():
    """Build the per-core Bass graph (identical on all 8 cores)."""
    nc = bass.Bass()

    # DRAM parameters (per-core shards, host-prepped layouts; all bf16
    # except the f32 z output).
    xT_d = nc.declare_dram_parameter("xT", [DIM, NQ], BF16, isOutput=False)
    yT_d = nc.declare_dram_parameter("yT", [DIM, NKV_SHARD], BF16, isOutput=False)
    # m: column slabs of M = Wq_scaled @ Wk^T: [do_chunk, d_in, 128],
    # slab j = M[:, j*128:(j+1)*128]
    m_d = nc.declare_dram_parameter("m", [8, DIM, 128], BF16, isOutput=False)
    wv_d = nc.declare_dram_parameter("wv", [DIM, DIM], BF16, isOutput=False)
    # transposed unnormalized output out'^T [d_out, nq], bf16
    outT_d = nc.declare_dram_parameter("outT", [DIM, NQ], BF16, isOutput=True)
    z_d = nc.declare_dram_parameter("zout", [1, NQ], F32, isOutput=True)

    with tile.TileContext(nc) as tc:
        # Long-lived pool: on-chip intermediates live to the end.
        L = tc.alloc_tile_pool(name="L", bufs=1)
        pm = tc.alloc_tile_pool(name="pm", bufs=1, space="PSUM")
        # Transient pools, released once consumed (LIFO release order).
        tx = tc.alloc_tile_pool(name="tx", bufs=1)  # xt + m slabs
        tw = tc.alloc_tile_pool(name="tw", bufs=1)  # wv

        # ---- HAM warm-up: ~24 dummy matmuls on a zeroed scratch tile run
        # during the otherwise-idle input-DMA window, flipping the PE clock
        # gate to full speed before the first real matmul arrives.
        # gpsimd clears its preamble ~1us before vector, so the warm-up
        # scratch is ready (and the first warm matmul issues) earlier.
        ws = L.tile([128, 512], BF16, name="warm", bufs=1)
        nc.gpsimd.memset(ws[:], 0.0)
        ones = L.tile([128, 1], BF16, name="ones", bufs=1)
        nc.gpsimd.memset(ones[:], 1.0)
        wps = pm.tile([128, 512], F32, name="wps", tag="mm", bufs=6)
        for w in range(24):
            nc.tensor.matmul(
                wps[:], lhsT=ws[:, 0:128], rhs=ws[:],
                start=(w == 0), stop=(w == 23),
            )

        # ---- Input DMAs, priority order.  P3 (V projection) runs first and
        # needs yT + Wv; its c-th contraction step gates only on chunk c.
        # P1 (x@M) needs m slab j + ALL of xT, so m[0] is issued before xT
        # and the remaining slabs after (they land during P3/P1).
        ytr = yT_d.rearrange("(c p) n -> c p n", p=128)
        wvr = wv_d.rearrange("(c p) n -> p c n", p=128)
        wv = tw.tile([128, 8, DIM], BF16, name="wv", bufs=1)
        ytc = []
        for c in range(8):
            t = L.tile([128, NKV_SHARD], BF16, name=f"yt{c}", tag="yt", bufs=8)
            nc.sync.dma_start(out=t[:], in_=ytr[c])
            ytc.append(t)
            nc.sync.dma_start(out=wv[:, c, :], in_=wvr[:, c, :])
        msl = [tx.tile([128, 8, 128], BF16, name=f"m{j}", tag="m", bufs=8)
               for j in range(8)]
        nc.sync.dma_start(
            out=msl[0][:], in_=m_d[0].rearrange("(c p) m -> p c m", p=128)
        )
        xt = tx.tile([128, 8, NQ], BF16, name="xt", bufs=1)
        nc.sync.dma_start(out=xt[:], in_=xT_d.rearrange("(c p) n -> p c n", p=128))
        for j in range(1, 8):
            nc.sync.dma_start(
                out=msl[j][:], in_=m_d[j].rearrange("(c p) m -> p c m", p=128)
            )

        # ---- P3: v[nkv, do] = sum_c yT[c-chunk, nkv].T @ Wv[c-chunk, do]
        # q-outer/c-inner everywhere: each psum bank completes (and its copy
        # starts) a full 8-matmul stream before the next bank stops, so the
        # 6-slot psum rotation never waits on a trailing copy.  Per-matmul
        # weight reload is free (measured: same 216ns cadence either way).
        vt = [L.tile([128, DIM], BF16, name=f"v{i}", tag="v", bufs=8) for i in range(8)]
        for i in range(8):  # nkv 128-tile
            for h in range(2):  # d_out 512-chunk
                ps = pm.tile([128, 512], F32, name=f"psv{i}_{h}", tag="mm", bufs=6)
                for c in range(8):  # d_in chunk (contraction)
                    nc.tensor.matmul(
                        ps[:],
                        lhsT=ytc[c][:, ds(i * 128, 128)],
                        rhs=wv[:, c, ds(h * 512, 512)],
                        start=(c == 0),
                        stop=(c == 7),
                    )
                nc.any.tensor_copy(vt[i][:, ds(h * 512, 512)], ps[:])
        tw.release()

        # ---- P1: tT[dm, nq] = sum_c M[c-chunk, dm-slab].T @ xT[c-chunk, nq]
        tt = [L.tile([128, NQ], BF16, name=f"t{j}", tag="tt", bufs=8) for j in range(8)]
        for j in range(8):  # dm slab
            for q in range(4):  # nq 512-chunk
                ps = pm.tile([128, 512], F32, name=f"pst{j}_{q}", tag="mm", bufs=6)
                for c in range(8):  # d_in chunk (contraction)
                    nc.tensor.matmul(
                        ps[:],
                        lhsT=msl[j][:, c, :],
                        rhs=xt[:, c, ds(q * 512, 512)],
                        start=(c == 0),
                        stop=(c == 7),
                    )
                nc.any.tensor_copy(tt[j][:, ds(q * 512, 512)], ps[:])
        tx.release()

        # ---- P4: expT[nkv, nq] = exp(sum_c yT[c,nkv].T @ tT[c,nq]) --------
        # The partial-Z add-tree (vector engine, otherwise idle) is emitted
        # pipelined with et production so stot is ready long before the
        # end-of-kernel Z matmuls.
        et = [L.tile([128, NQ], BF16, name=f"e{i}", tag="et", bufs=8) for i in range(8)]
        tz = tc.alloc_tile_pool(name="tz", bufs=1)
        # bf16 throughout: 2x DVE throughput, and keeps the Z ones-matmul a
        # single-pass bf16 matmul instead of a two-pass fp32 one.  Z error
        # from bf16 partials averages down over 128 all-positive partition
        # sums (~0.05% on Z).
        s0 = [tz.tile([128, NQ], BF16, name=f"es0_{h}", tag="es", bufs=5)
              for h in range(4)]
        s1 = [tz.tile([128, NQ], BF16, name=f"es1_{h}", tag="es2", bufs=2)
              for h in range(2)]
        stot = tz.tile([128, NQ], BF16, name="estot", tag="es", bufs=5)
        for i in range(8):  # nkv 128-tile
            for q in range(4):  # nq 512-chunk
                ps = pm.tile([128, 512], F32, name=f"pse{i}_{q}", tag="mm", bufs=6)
                for c in range(8):  # dm chunk (contraction)
                    nc.tensor.matmul(
                        ps[:],
                        lhsT=ytc[c][:, ds(i * 128, 128)],
                        rhs=tt[c][:, ds(q * 512, 512)],
                        start=(c == 0),
                        stop=(c == 7),
                    )
                nc.scalar.activation(
                    et[i][:, ds(q * 512, 512)],
                    ps[:],
                    mybir.ActivationFunctionType.Exp,
                )
            if i % 2 == 1:  # pair (i-1, i) complete -> level-0 add
                nc.vector.tensor_add(s0[i // 2][:], et[i - 1][:], et[i][:])
            if i == 3:
                nc.vector.tensor_add(s1[0][:], s0[0][:], s0[1][:])
            if i == 7:
                nc.vector.tensor_add(s1[1][:], s0[2][:], s0[3][:])
                nc.vector.tensor_add(stot[:], s1[0][:], s1[1][:])

        # ---- P7: out'^T[do, nq] = sum_i v[i-tile, do-slab].T @ expT[i, nq]
        # The Z ones-matmuls (P5) are emitted after group d=0: stot is ready
        # by then, and the whole Z chain (4 bf16 matmuls + copies + DMA)
        # hides under the remaining P7 groups instead of extending the tail.
        for d in range(8):  # d_out 128-tile
            for q in range(4):  # nq 512-chunk
                ps = pm.tile([128, 512], F32, name=f"pso{d}_{q}", tag="mm", bufs=6)
                for i in range(8):  # nkv contraction
                    nc.tensor.matmul(
                        ps[:],
                        lhsT=vt[i][:, ds(d * 128, 128)],
                        rhs=et[i][:, ds(q * 512, 512)],
                        start=(i == 0),
                        stop=(i == 7),
                    )
                ob = L.tile([128, 512], BF16, name=f"o{d}_{q}", tag="o", bufs=4)
                nc.any.tensor_copy(ob[:], ps[:])
                # last group: spread descriptor-gen over two engines so the
                # final DMAs don't serialize ~600ns apiece on the sync queue
                eng = nc.gpsimd if (d == 7 and q % 2 == 1) else nc.sync
                eng.dma_start(
                    out=outT_d[ds(d * 128, 128), ds(q * 512, 512)], in_=ob[:]
                )
            if d < 4:
                # ---- P5: Z[nq] = sum_nkv expT[nkv, nq] = ones.T @ stot ---
                # one chunk per P7 group: each psz bank/copy gets a full
                # group (~7us) of slack, so the PE never waits on the chain
                q5 = d
                psz = pm.tile([1, 512], F32, name=f"psz{q5}", tag="z", bufs=2)
                nc.tensor.matmul(
                    psz[:],
                    lhsT=ones[:],
                    rhs=stot[:, ds(q5 * 512, 512)],
                    start=True,
                    stop=True,
                )
                zrow = L.tile([1, 512], F32, name=f"zrow{q5}", tag="zrow", bufs=2)
                nc.any.tensor_copy(zrow[:], psz[:])
                nc.sync.dma_start(out=z_d[0:1, ds(q5 * 512, 512)], in_=zrow[:])
        tz.release()
        pm.release()
        L.release()

    _split_sync_waits(nc)
    return nc


_NC_CACHE = {}


def _get_nc():
    if "nc" not in _NC_CACHE:
        _NC_CACHE["nc"] = build_attention_nc()
    return _NC_CACHE["nc"]


def make_in_maps(x, y, Wq, Wkv):
    """Host-side sharding + layout prep. Returns in_maps for cores 0-7."""
    scale = DIM ** (-0.5)
    wq_s = np.asarray(Wq, np.float32) * scale
    wkv = np.asarray(Wkv, np.float32)
    wk = wkv[:, :DIM]
    wv = wkv[:, DIM:].astype(NP_BF16)
    # M = Wq_scaled @ Wk^T, f32 accumulate then bf16; column slabs [8, DIM, 128]
    m = (wq_s @ wk.T).astype(NP_BF16)
    m_slabs = np.ascontiguousarray(m.reshape(DIM, 8, 128).transpose(1, 0, 2))

    x = np.asarray(x, np.float32)
    y = np.asarray(y, np.float32)
    in_maps = []
    for core in range(N_CORES):
        b, s = divmod(core, 2)
        xT = np.ascontiguousarray(x[b].T).astype(NP_BF16)
        yT = np.ascontiguousarray(
            y[b, s * NKV_SHARD : (s + 1) * NKV_SHARD, :].T
        ).astype(NP_BF16)
        in_maps.append({"xT": xT, "yT": yT, "m": m_slabs, "wv": wv})
    return in_maps


def run_sharded(x, y, Wq, Wkv, trace=False, tmpdir=None):
    """Run the SPMD kernel; returns (full_output, BassKernelResults)."""
    nc = _get_nc()
    in_maps = make_in_maps(x, y, Wq, Wkv)
    try:
        res = run_bass_kernel_spmd(
            nc, in_maps, core_ids=list(range(N_CORES)), trace=trace, tmpdir=tmpdir
        )
    except Exception:
        # one retry: transient NRT device states (e.g. a previous crashed
        # load) usually clear on the next attempt
        res = run_bass_kernel_spmd(
            nc, in_maps, core_ids=list(range(N_CORES)), trace=trace, tmpdir=tmpdir
        )
    out = np.empty((B, NQ, DIM), np.float32)
    for b in range(B):
        r0, r1 = res.results[2 * b], res.results[2 * b + 1]
        num = r0["outT"].astype(np.float32) + r1["outT"].astype(np.float32)
        z = (r0["zout"] + r1["zout"]).reshape(NQ)
        out[b] = (num / z[None, :]).T
    return out, res


def kernel(x, y, Wq, Wkv):
    out, _ = run_sharded(x, y, Wq, Wkv)
    return out


# revision 18
# speedup vs baseline: 1.1889x; 1.1889x over previous
"""Self-contained Trainium2 Bass kernel for single-head full-dim attention.

Reference computation (fp32 jax):
    q  = x @ Wq                      # [B, Nq, D]
    kv = y @ Wkv                     # [B, Nkv, 2D] -> k, v
    attn = softmax(q * D^-0.5 @ k^T) # [B, Nq, Nkv]
    out  = attn @ v                  # [B, Nq, D]
with B=4, Nq=Nkv=2048, D=1024.

Distribution: data parallel over 8 NeuronCores, shard = (batch b,
kv-half s).  Each core handles all 2048 queries of its batch against
its 1024 keys, producing the UNNORMALIZED output block
out'_s = exp(S_s) @ v_s and the partial softmax denominator
Z_s = sum_k exp(S_s).  The host combines the two halves:
out = (out'_0 + out'_1) / (Z_0 + Z_1).  No collectives.

Key algebraic trick: the Q and K projections are folded into a single
host-precomputed matrix M = (Wq * D^-0.5) @ Wk^T, so
    scores = (x @ M) @ y^T.
This removes the K projection entirely and de-duplicates the Q
projection across the core pair "for free": per-core flops hit the
ideal total/8 (15.0 GF vs 17.2 GF for the q/k-projection layout).

Layouts: everything on-chip is computed transposed ([feature, token])
so the TensorEngine contracts along partitions without any on-chip
transposes; the unnormalized output is produced transposed
([d_out, nq]) which lets the stationary operand be a v-tile and the
moving operand stream full 512-wide nq chunks.  All matmul operands
bf16 (fp32 PSUM accumulation); the out' result is stored bf16 (the
2e-2 rel tolerance dwarfs bf16 rounding).  exp runs without
max-subtraction (scores ~ N(0,1) by construction) on the scalar
engine; Z is a ones-vector matmul emitted after P7 so its serial
chain hides behind the output-DMA drain.
"""

import numpy as np
import ml_dtypes

import concourse.bass as bass
import concourse.mybir as mybir
import concourse.tile as tile
from concourse.bass import ds
from concourse.bass_utils import run_bass_kernel_spmd

DIM = 1024
B = 4
NQ = 2048
NKV = 2048
N_CORES = 8
NKV_SHARD = 1024  # keys per core

BF16 = mybir.dt.bfloat16
F32 = mybir.dt.float32
NP_BF16 = ml_dtypes.bfloat16


def _split_sync_waits(nc, max_waits: int = 1):
    """walrus in this toolchain rejects instructions carrying more than one
    sem wait ("Too many sync wait commands").  Hoist extra waits onto
    preceding same-engine NOPs: the engine dispatches in order, so waiting
    just before the instruction is semantically identical (at worst it
    delays issue slightly)."""
    import bass_rust as _bass_rust

    for f in nc.m.functions:
        for bb in f.blocks:
            insts = list(bb.instructions)
            out = []
            changed = False
            for inst in insts:
                si = getattr(inst, "sync_info", None)
                waits = list(si.on_wait) if si is not None and si.on_wait else []
                if len(waits) > max_waits:
                    changed = True
                    extra, keep = waits[:-max_waits], waits[-max_waits:]
                    for k in range(0, len(extra), max_waits):
                        nop = mybir.InstNoOp(
                            name=f"{inst.name}_sw{k}", engine=inst.engine,
                            ins=[], outs=[],
                        )
                        nop.sync_info = _bass_rust.SyncInfo(
                            on_wait=extra[k : k + max_waits], on_update=[]
                        )
                        out.append(nop)
                    si.on_wait = keep
                    inst.sync_info = si
                out.append(inst)
            if changed:
                bb.instructions = out


def build_attention_nc():
    """Build the per-core Bass graph (identical on all 8 cores)."""
    nc = bass.Bass()

    # DRAM parameters (per-core shards, host-prepped layouts; all bf16
    # except the f32 z output).
    xT_d = nc.declare_dram_parameter("xT", [DIM, NQ], BF16, isOutput=False)
    yT_d = nc.declare_dram_parameter("yT", [DIM, NKV_SHARD], BF16, isOutput=False)
    # m: column slabs of M = Wq_scaled @ Wk^T: [do_chunk, d_in, 128],
    # slab j = M[:, j*128:(j+1)*128]
    m_d = nc.declare_dram_parameter("m", [8, DIM, 128], BF16, isOutput=False)
    wv_d = nc.declare_dram_parameter("wv", [DIM, DIM], BF16, isOutput=False)
    # transposed unnormalized output out'^T [d_out, nq], bf16
    outT_d = nc.declare_dram_parameter("outT", [DIM, NQ], BF16, isOutput=True)
    z_d = nc.declare_dram_parameter("zout", [1, NQ], F32, isOutput=True)

    with tile.TileContext(nc) as tc:
        # Long-lived pool: on-chip intermediates live to the end.
        L = tc.alloc_tile_pool(name="L", bufs=1)
        pm = tc.alloc_tile_pool(name="pm", bufs=1, space="PSUM")
        # Transient pools, released once consumed (LIFO release order).
        tx = tc.alloc_tile_pool(name="tx", bufs=1)  # xt + m slabs
        tw = tc.alloc_tile_pool(name="tw", bufs=1)  # wv

        # ---- HAM warm-up: ~24 dummy matmuls on a zeroed scratch tile run
        # during the otherwise-idle input-DMA window, flipping the PE clock
        # gate to full speed before the first real matmul arrives.
        # gpsimd clears its preamble ~1us before vector, so the warm-up
        # scratch is ready (and the first warm matmul issues) earlier.
        ws = L.tile([128, 512], BF16, name="warm", bufs=1)
        nc.gpsimd.memset(ws[:], 0.0)
        ones = L.tile([128, 1], BF16, name="ones", bufs=1)
        nc.gpsimd.memset(ones[:], 1.0)
        wps = pm.tile([128, 512], F32, name="wps", tag="mm", bufs=6)
        for w in range(24):
            nc.tensor.matmul(
                wps[:], lhsT=ws[:, 0:128], rhs=ws[:],
                start=(w == 0), stop=(w == 23),
            )

        # ---- Input DMAs, priority order.  P3 (V projection) runs first and
        # needs yT + Wv; its c-th contraction step gates only on chunk c.
        # P1 (x@M) needs m slab j + ALL of xT, so m[0] is issued before xT
        # and the remaining slabs after (they land during P3/P1).
        ytr = yT_d.rearrange("(c p) n -> c p n", p=128)
        wvr = wv_d.rearrange("(c p) n -> p c n", p=128)
        wv = tw.tile([128, 8, DIM], BF16, name="wv", bufs=1)
        ytc = []
        for c in range(8):
            t = L.tile([128, NKV_SHARD], BF16, name=f"yt{c}", tag="yt", bufs=8)
            nc.sync.dma_start(out=t[:], in_=ytr[c])
            ytc.append(t)
            nc.sync.dma_start(out=wv[:, c, :], in_=wvr[:, c, :])
        msl = [tx.tile([128, 8, 128], BF16, name=f"m{j}", tag="m", bufs=8)
               for j in range(8)]
        nc.sync.dma_start(
            out=msl[0][:], in_=m_d[0].rearrange("(c p) m -> p c m", p=128)
        )
        xt = tx.tile([128, 8, NQ], BF16, name="xt", bufs=1)
        nc.sync.dma_start(out=xt[:], in_=xT_d.rearrange("(c p) n -> p c n", p=128))
        for j in range(1, 8):
            nc.sync.dma_start(
                out=msl[j][:], in_=m_d[j].rearrange("(c p) m -> p c m", p=128)
            )

        # ---- P3: v[nkv, do] = sum_c yT[c-chunk, nkv].T @ Wv[c-chunk, do]
        # q-outer/c-inner everywhere: each psum bank completes (and its copy
        # starts) a full 8-matmul stream before the next bank stops, so the
        # 6-slot psum rotation never waits on a trailing copy.  Per-matmul
        # weight reload is free (measured: same 216ns cadence either way).
        vt = [L.tile([128, DIM], BF16, name=f"v{i}", tag="v", bufs=8) for i in range(8)]
        for i in range(8):  # nkv 128-tile
            for h in range(2):  # d_out 512-chunk
                ps = pm.tile([128, 512], F32, name=f"psv{i}_{h}", tag="mm", bufs=6)
                for c in range(8):  # d_in chunk (contraction)
                    nc.tensor.matmul(
                        ps[:],
                        lhsT=ytc[c][:, ds(i * 128, 128)],
                        rhs=wv[:, c, ds(h * 512, 512)],
                        start=(c == 0),
                        stop=(c == 7),
                    )
                nc.any.tensor_copy(vt[i][:, ds(h * 512, 512)], ps[:])
        tw.release()

        # ---- P1: tT[dm, nq] = sum_c M[c-chunk, dm-slab].T @ xT[c-chunk, nq]
        tt = [L.tile([128, NQ], BF16, name=f"t{j}", tag="tt", bufs=8) for j in range(8)]
        for j in range(8):  # dm slab
            for q in range(4):  # nq 512-chunk
                ps = pm.tile([128, 512], F32, name=f"pst{j}_{q}", tag="mm", bufs=6)
                for c in range(8):  # d_in chunk (contraction)
                    nc.tensor.matmul(
                        ps[:],
                        lhsT=msl[j][:, c, :],
                        rhs=xt[:, c, ds(q * 512, 512)],
                        start=(c == 0),
                        stop=(c == 7),
                    )
                nc.any.tensor_copy(tt[j][:, ds(q * 512, 512)], ps[:])
        tx.release()

        # ---- P4: expT[nkv, nq] = exp(sum_c yT[c,nkv].T @ tT[c,nq]) --------
        # The partial-Z add-tree (vector engine, otherwise idle) is emitted
        # pipelined with et production so stot is ready long before the
        # end-of-kernel Z matmuls.
        et = [L.tile([128, NQ], BF16, name=f"e{i}", tag="et", bufs=8) for i in range(8)]
        tz = tc.alloc_tile_pool(name="tz", bufs=1)
        # bf16 throughout: 2x DVE throughput, and keeps the Z ones-matmul a
        # single-pass bf16 matmul instead of a two-pass fp32 one.  Z error
        # from bf16 partials averages down over 128 all-positive partition
        # sums (~0.05% on Z).
        s0 = [tz.tile([128, NQ], BF16, name=f"es0_{h}", tag="es", bufs=5)
              for h in range(4)]
        s1 = [tz.tile([128, NQ], BF16, name=f"es1_{h}", tag="es2", bufs=2)
              for h in range(2)]
        stot = tz.tile([128, NQ], BF16, name="estot", tag="es", bufs=5)
        for i in range(8):  # nkv 128-tile
            for q in range(4):  # nq 512-chunk
                ps = pm.tile([128, 512], F32, name=f"pse{i}_{q}", tag="mm", bufs=6)
                for c in range(8):  # dm chunk (contraction)
                    nc.tensor.matmul(
                        ps[:],
                        lhsT=ytc[c][:, ds(i * 128, 128)],
                        rhs=tt[c][:, ds(q * 512, 512)],
                        start=(c == 0),
                        stop=(c == 7),
                    )
                nc.scalar.activation(
                    et[i][:, ds(q * 512, 512)],
                    ps[:],
                    mybir.ActivationFunctionType.Exp,
                )
            if i % 2 == 1:  # pair (i-1, i) complete -> level-0 add
                nc.vector.tensor_add(s0[i // 2][:], et[i - 1][:], et[i][:])
            if i == 3:
                nc.vector.tensor_add(s1[0][:], s0[0][:], s0[1][:])
            if i == 7:
                nc.vector.tensor_add(s1[1][:], s0[2][:], s0[3][:])
                nc.vector.tensor_add(stot[:], s1[0][:], s1[1][:])

        # ---- P7: out'^T[do, nq] = sum_i v[i-tile, do-slab].T @ expT[i, nq]
        # The Z ones-matmuls (P5) are emitted after group d=0: stot is ready
        # by then, and the whole Z chain (4 bf16 matmuls + copies + DMA)
        # hides under the remaining P7 groups instead of extending the tail.
        for d in range(8):  # d_out 128-tile
            for q in range(4):  # nq 512-chunk
                ps = pm.tile([128, 512], F32, name=f"pso{d}_{q}", tag="mm", bufs=6)
                for i in range(8):  # nkv contraction
                    nc.tensor.matmul(
                        ps[:],
                        lhsT=vt[i][:, ds(d * 128, 128)],
                        rhs=et[i][:, ds(q * 512, 512)],
                        start=(i == 0),
                        stop=(i == 7),
                    )
                ob = L.tile([128, 512], BF16, name=f"o{d}_{q}", tag="o", bufs=4)
                nc.any.tensor_copy(ob[:], ps[:])
                # last group: one DMA-issue engine per chunk so descriptor
                # generation (~600ns apiece) never serializes the drain
                if d == 7:
                    eng = [nc.sync, nc.gpsimd, nc.sync, nc.scalar][q]
                else:
                    eng = nc.sync
                eng.dma_start(
                    out=outT_d[ds(d * 128, 128), ds(q * 512, 512)], in_=ob[:]
                )
            if d < 4:
                # ---- P5: Z[nq] = sum_nkv expT[nkv, nq] = ones.T @ stot ---
                # one chunk per P7 group: each psz bank/copy gets a full
                # group (~7us) of slack, so the PE never waits on the chain
                q5 = d
                psz = pm.tile([1, 512], F32, name=f"psz{q5}", tag="z", bufs=2)
                nc.tensor.matmul(
                    psz[:],
                    lhsT=ones[:],
                    rhs=stot[:, ds(q5 * 512, 512)],
                    start=True,
                    stop=True,
                )
                zrow = L.tile([1, 512], F32, name=f"zrow{q5}", tag="zrow", bufs=2)
                nc.any.tensor_copy(zrow[:], psz[:])
                nc.sync.dma_start(out=z_d[0:1, ds(q5 * 512, 512)], in_=zrow[:])
        tz.release()
        pm.release()
        L.release()

    _split_sync_waits(nc)
    return nc


_NC_CACHE = {}


def _get_nc():
    if "nc" not in _NC_CACHE:
        _NC_CACHE["nc"] = build_attention_nc()
    return _NC_CACHE["nc"]


def make_in_maps(x, y, Wq, Wkv):
    """Host-side sharding + layout prep. Returns in_maps for cores 0-7."""
    scale = DIM ** (-0.5)
    wq_s = np.asarray(Wq, np.float32) * scale
    wkv = np.asarray(Wkv, np.float32)
    wk = wkv[:, :DIM]
    wv = wkv[:, DIM:].astype(NP_BF16)
    # M = Wq_scaled @ Wk^T, f32 accumulate then bf16; column slabs [8, DIM, 128]
    m = (wq_s @ wk.T).astype(NP_BF16)
    m_slabs = np.ascontiguousarray(m.reshape(DIM, 8, 128).transpose(1, 0, 2))

    x = np.asarray(x, np.float32)
    y = np.asarray(y, np.float32)
    in_maps = []
    for core in range(N_CORES):
        b, s = divmod(core, 2)
        xT = np.ascontiguousarray(x[b].T).astype(NP_BF16)
        yT = np.ascontiguousarray(
            y[b, s * NKV_SHARD : (s + 1) * NKV_SHARD, :].T
        ).astype(NP_BF16)
        in_maps.append({"xT": xT, "yT": yT, "m": m_slabs, "wv": wv})
    return in_maps


def run_sharded(x, y, Wq, Wkv, trace=False, tmpdir=None):
    """Run the SPMD kernel; returns (full_output, BassKernelResults)."""
    nc = _get_nc()
    in_maps = make_in_maps(x, y, Wq, Wkv)
    try:
        res = run_bass_kernel_spmd(
            nc, in_maps, core_ids=list(range(N_CORES)), trace=trace, tmpdir=tmpdir
        )
    except Exception:
        # one retry: transient NRT device states (e.g. a previous crashed
        # load) usually clear on the next attempt
        res = run_bass_kernel_spmd(
            nc, in_maps, core_ids=list(range(N_CORES)), trace=trace, tmpdir=tmpdir
        )
    out = np.empty((B, NQ, DIM), np.float32)
    for b in range(B):
        r0, r1 = res.results[2 * b], res.results[2 * b + 1]
        num = r0["outT"].astype(np.float32) + r1["outT"].astype(np.float32)
        z = (r0["zout"] + r1["zout"]).reshape(NQ)
        out[b] = (num / z[None, :]).T
    return out, res


def kernel(x, y, Wq, Wkv):
    out, _ = run_sharded(x, y, Wq, Wkv)
    return out
